# revision 3
# baseline (speedup 1.0000x reference)
"""ArcticMoeBlock on 8 TRN2 NeuronCores — expert-parallel Bass kernel.

Reference computation (B=1, S=2048, H=1024, F=4096, E=8, TOPK=2):
    router_logits = x @ Wg                                   [S, E]  (fp32)
    top-2 softmax -> combine[s, e] (nonzero only for the 2 selected experts)
    per expert e: y_e = (silu(x @ W1[e]) * (x @ W3[e])) @ W2[e]
    final[s, :] = sum_e combine[s, e] * y_e[s, :]

Sharding: expert-parallel. Core e holds W1/W3/W2 of expert e (bf16,
host-packed for contiguous DMA), the token batch is replicated (passed
pre-transposed as xT so no on-device transposes are needed). Every core
computes the full fp32 router, selects its own combine column via a
one-hot vector, scales its expert output, and an AllReduce over the 8
cores produces the final output on every core.

Matmul layout notes (out = lhsT.T @ rhs, lhsT stationary [K<=128, M<=128]):
  h1T[f, s] = sum_h W1[h, f] xT[h, s]   lhsT = W1 tile (h-part), rhs = xT
  y[s, h]   = sum_f g[s, f] W2[f, h]    lhsT = gT tile (f-part), rhs = W2
so activations stay in [feature, token] layout until the W2 matmul flips
them back to natural [token, feature] — zero transposes anywhere.
"""

import numpy as np
import ml_dtypes

import concourse.bass as bass
import concourse.mybir as mybir
import concourse.tile as tile
from concourse import bacc
from concourse.bass_utils import run_bass_kernel_spmd

AF = mybir.ActivationFunctionType
ALU = mybir.AluOpType
AX = mybir.AxisListType
F32 = mybir.dt.float32
BF16 = mybir.dt.bfloat16

P = 128        # partition count
NF = 512       # matmul moving-operand chunk (one PSUM bank in fp32)
N_CORES = 8

# Set by test harness to capture profile info; harmless otherwise.
TRACE = False
LAST_RESULT = None

_compiled = {}


def _build(S, H, F, E, n_cores, s_blk):
    KH = H // P          # contraction tiles for W1/W3 (over hidden dim)
    MF = F // P          # f-tiles (output partitions of W1/W3, contraction of W2)
    NS = S // P          # s-tiles over the full sequence
    NBLK = S // s_blk    # token blocks
    MSB = s_blk // P     # s-tiles per block
    SFB = s_blk // NF    # 512-wide s-chunks per block (W1/W3 rhs)
    NH = H // NF         # 512-wide h-chunks (W2 rhs)
    NSE = NS * E
    assert NSE <= NF and MSB <= 8 and 2 * SFB <= 8

    nc = bacc.Bacc("TRN2", target_bir_lowering=False, debug=False,
                   num_devices=n_cores)

    xt32_e = nc.dram_tensor("xt_f32", [H, S], F32, kind="ExternalInput")
    xtbf_e = nc.dram_tensor("xt_bf16", [H, S], BF16, kind="ExternalInput")
    wg_e = nc.dram_tensor("wg", [H, E], F32, kind="ExternalInput")
    w1t_e = nc.dram_tensor("w1t", [MF, P, H], BF16, kind="ExternalInput")
    w3t_e = nc.dram_tensor("w3t", [MF, P, H], BF16, kind="ExternalInput")
    w2_e = nc.dram_tensor("w2", [F, H], BF16, kind="ExternalInput")
    esel_e = nc.dram_tensor("esel", [P, E], F32, kind="ExternalInput")
    outf_e = nc.dram_tensor("out_final", [S, H], F32, kind="ExternalOutput")
    outl_e = nc.dram_tensor("out_logits", [S, E], F32, kind="ExternalOutput")

    with tile.TileContext(nc) as tc:
        with (
            tc.tile_pool(name="persist", bufs=1) as pp,
            tc.tile_pool(name="xtbp", bufs=KH) as xtbp,
            tc.tile_pool(name="xfp", bufs=6) as xfp,
            tc.tile_pool(name="wgp", bufs=KH) as wgp,
            tc.tile_pool(name="w13p", bufs=3) as w13p,
            tc.tile_pool(name="w2p", bufs=4) as w2p,
            tc.tile_pool(name="gp", bufs=MF) as gp,
            tc.tile_pool(name="silup", bufs=4) as silup,
            tc.tile_pool(name="obp", bufs=4) as obp,
            tc.tile_pool(name="psp", bufs=8, space="PSUM") as psp,
            tc.tile_pool(name="dramp", bufs=1, space="DRAM") as dramp,
        ):
            # ---------------- router (fp32) ----------------
            # One PSUM bank per s-tile: a start=True matmul clears the whole
            # bank's accumulation state, so groups must not share a bank.
            wgs = []
            for k in range(KH):
                wgk = wgp.tile([P, E], F32, tag="wgk", name=f"wgk{k}")
                nc.sync.dma_start(out=wgk[:], in_=wg_e[k * P:(k + 1) * P, :])
                wgs.append(wgk)
            logits_sb = pp.tile([P, NSE], F32)
            for m in range(NS):
                ps_m = psp.tile([P, E], F32, tag="ps", name=f"psl{m}")
                for k in range(KH):
                    xf = xfp.tile([P, P], F32, tag="xf", name=f"xf_{m}_{k}")
                    nc.sync.dma_start(
                        out=xf[:],
                        in_=xt32_e[k * P:(k + 1) * P, m * P:(m + 1) * P])
                    nc.tensor.matmul(ps_m[:], xf[:], wgs[k][:],
                                     start=(k == 0), stop=(k == KH - 1))
                nc.vector.tensor_copy(logits_sb[:, m * E:(m + 1) * E], ps_m[:])
            nc.sync.dma_start(
                out=outl_e.rearrange("(m p) e -> p m e", p=P),
                in_=logits_sb[:].rearrange("p (m e) -> p m e", e=E),
            )

            # combine weights: top-2 softmax, select this core's expert col
            esel_sb = pp.tile([P, E], F32)
            nc.sync.dma_start(out=esel_sb[:], in_=esel_e[:])

            L3 = logits_sb[:].rearrange("p (m e) -> p m e", e=E)
            m1 = pp.tile([P, NS], F32)
            nc.vector.reduce_max(m1[:], L3, axis=AX.X)
            mask = pp.tile([P, NSE], F32)
            mask3 = mask[:].rearrange("p (m e) -> p m e", e=E)
            m1b = m1[:].unsqueeze(2).broadcast_to([P, NS, E])
            nc.vector.tensor_tensor(mask3, L3, m1b, op=ALU.is_equal)
            lm = pp.tile([P, NSE], F32)
            lm3 = lm[:].rearrange("p (m e) -> p m e", e=E)
            # (mask * -1e30) + logits: masked-out argmax -> -inf
            nc.vector.scalar_tensor_tensor(
                lm3, mask3, -1e30, L3, op0=ALU.mult, op1=ALU.add)
            m2 = pp.tile([P, NS], F32)
            nc.vector.reduce_max(m2[:], lm3, axis=AX.X)
            dm = pp.tile([P, NS], F32)
            nc.vector.tensor_sub(dm[:], m1[:], m2[:])
            w1w = pp.tile([P, NS], F32)
            nc.scalar.activation(w1w[:], dm[:], AF.Sigmoid)
            w2w = pp.tile([P, NS], F32)
            nc.vector.tensor_scalar(w2w[:], w1w[:], -1.0, 1.0,
                                    op0=ALU.mult, op1=ALU.add)
            lesel = pp.tile([P, NSE], F32)
            le3 = lesel[:].rearrange("p (m e) -> p m e", e=E)
            eselb = esel_sb[:].unsqueeze(1).broadcast_to([P, NS, E])
            nc.vector.tensor_tensor(le3, L3, eselb, op=ALU.mult)
            le = pp.tile([P, NS], F32)
            nc.vector.reduce_sum(le[:], le3, axis=AX.X)
            eq1 = pp.tile([P, NS], F32)
            nc.vector.tensor_tensor(eq1[:], le[:], m1[:], op=ALU.is_equal)
            eq2 = pp.tile([P, NS], F32)
            nc.vector.tensor_tensor(eq2[:], le[:], m2[:], op=ALU.is_equal)
            cq1 = pp.tile([P, NS], F32)
            nc.vector.tensor_tensor(cq1[:], eq1[:], w1w[:], op=ALU.mult)
            cq2 = pp.tile([P, NS], F32)
            nc.vector.tensor_tensor(cq2[:], eq2[:], w2w[:], op=ALU.mult)
            c_sb = pp.tile([P, NS], F32)
            nc.vector.tensor_add(c_sb[:], cq1[:], cq2[:])

            # ---------------- resident bf16 xT ----------------
            xtb = []
            for k in range(KH):
                xk = xtbp.tile([P, S], BF16, tag="xtb", name=f"xtb{k}")
                nc.sync.dma_start(out=xk[:], in_=xtbf_e[k * P:(k + 1) * P, :])
                xtb.append(xk)

            ar_in = dramp.tile([S, H], F32)
            ar_out = dramp.tile([S, H], F32, addr_space="Shared")

            # ---------------- expert FFN over token blocks ----------------
            for b in range(NBLK):
                bs0 = b * s_blk
                # h1T/h3T/gT for this block, f-tile by f-tile
                gms = []
                for m in range(MF):
                    w1m = w13p.tile([P, H], BF16, tag="w1m", name=f"w1m_{b}_{m}")
                    nc.sync.dma_start(out=w1m[:], in_=w1t_e[m, :, :])
                    w3m = w13p.tile([P, H], BF16, tag="w3m", name=f"w3m_{b}_{m}")
                    nc.sync.dma_start(out=w3m[:], in_=w3t_e[m, :, :])
                    ph1 = [psp.tile([P, NF], F32, tag="ps", name=f"ph1_{b}_{m}_{sf}")
                           for sf in range(SFB)]
                    ph3 = [psp.tile([P, NF], F32, tag="ps", name=f"ph3_{b}_{m}_{sf}")
                           for sf in range(SFB)]
                    for k in range(KH):
                        st, sp = (k == 0), (k == KH - 1)
                        for sf in range(SFB):
                            rhs = xtb[k][:, bs0 + sf * NF: bs0 + (sf + 1) * NF]
                            nc.tensor.matmul(ph1[sf][:], w1m[:, k * P:(k + 1) * P],
                                             rhs, start=st, stop=sp)
                        for sf in range(SFB):
                            rhs = xtb[k][:, bs0 + sf * NF: bs0 + (sf + 1) * NF]
                            nc.tensor.matmul(ph3[sf][:], w3m[:, k * P:(k + 1) * P],
                                             rhs, start=st, stop=sp)
                    gm = gp.tile([P, s_blk], BF16, tag="gm", name=f"gm_{b}_{m}")
                    for sf in range(SFB):
                        silu_t = silup.tile([P, NF], F32, tag="silu",
                                            name=f"silu_{b}_{m}_{sf}")
                        nc.scalar.activation(silu_t[:], ph1[sf][:], AF.Silu)
                        nc.vector.tensor_tensor(
                            gm[:, sf * NF:(sf + 1) * NF],
                            silu_t[:], ph3[sf][:], op=ALU.mult)
                    gms.append(gm)

                # y[s, h] = gT.T @ W2, k-outer so W2 streams exactly once
                for n in range(NH):
                    pso = [psp.tile([P, NF], F32, tag="ps", name=f"pso_{b}_{n}_{ms}")
                           for ms in range(MSB)]
                    for k in range(MF):
                        w2t = w2p.tile([P, NF], BF16, tag="w2t",
                                       name=f"w2t_{b}_{n}_{k}")
                        nc.sync.dma_start(
                            out=w2t[:],
                            in_=w2_e[k * P:(k + 1) * P, n * NF:(n + 1) * NF])
                        st, sp = (k == 0), (k == MF - 1)
                        for ms in range(MSB):
                            nc.tensor.matmul(
                                pso[ms][:],
                                gms[k][:, ms * P:(ms + 1) * P],
                                w2t[:], start=st, stop=sp)
                    for ms in range(MSB):
                        gs = b * MSB + ms
                        ob = obp.tile([P, NF], F32, tag="ob",
                                      name=f"ob_{b}_{n}_{ms}")
                        nc.vector.tensor_scalar_mul(
                            ob[:], pso[ms][:], c_sb[:, gs:gs + 1])
                        nc.sync.dma_start(
                            out=ar_in[bs0 + ms * P: bs0 + (ms + 1) * P,
                                      n * NF:(n + 1) * NF],
                            in_=ob[:])

            # ---------------- combine experts ----------------
            nc.gpsimd.collective_compute(
                "AllReduce",
                ALU.add,
                replica_groups=[list(range(n_cores))],
                ins=[ar_in[:]],
                outs=[ar_out[:]],
            )
            nc.sync.dma_start(out=outf_e[:], in_=ar_out[:])

    nc.compile()
    return nc


def _get_compiled(S, H, F, E, n_cores, s_blk):
    key = (S, H, F, E, n_cores, s_blk)
    if key not in _compiled:
        _compiled[key] = _build(*key)
    return _compiled[key]


def _pack_w13(w, H, F):
    # [H, F] -> [F//P, P, H] with w_packed[m, p, k*P+f] = w[k*P+p, m*P+f]
    return np.ascontiguousarray(
        w.astype(ml_dtypes.bfloat16)
        .reshape(H // P, P, F // P, P)
        .transpose(2, 1, 0, 3)
        .reshape(F // P, P, H))


def kernel(x, Wg, W1, W3, W2, s_blk=1024):
    global LAST_RESULT
    x = np.asarray(x)
    Wg = np.asarray(Wg, dtype=np.float32)
    W1 = np.asarray(W1)
    W3 = np.asarray(W3)
    W2 = np.asarray(W2)
    B, S, H = x.shape
    E = Wg.shape[1]
    F = W1.shape[2]
    assert B == 1 and E == N_CORES

    xt = np.ascontiguousarray(x.reshape(S, H).T.astype(np.float32))
    xt_bf = xt.astype(ml_dtypes.bfloat16)

    nc = _get_compiled(S, H, F, E, N_CORES, s_blk)

    in_maps = []
    for e in range(N_CORES):
        esel = np.zeros((P, E), np.float32)
        esel[:, e] = 1.0
        in_maps.append({
            "xt_f32": xt,
            "xt_bf16": xt_bf,
            "wg": Wg,
            "w1t": _pack_w13(W1[e], H, F),
            "w3t": _pack_w13(W3[e], H, F),
            "w2": np.ascontiguousarray(W2[e].astype(ml_dtypes.bfloat16)),
            "esel": esel,
        })

    if TRACE:
        import profhook  # noqa: F401  (injects the axon NTFF hook)
    res = run_bass_kernel_spmd(nc, in_maps, core_ids=list(range(N_CORES)),
                               trace=TRACE)
    LAST_RESULT = res
    final = np.asarray(res.results[0]["out_final"],
                       dtype=np.float32).reshape(B, S, H)
    logits = np.asarray(res.results[0]["out_logits"],
                        dtype=np.float32).reshape(B, S, E)
    return final, logits


# revision 6
# speedup vs baseline: 1.0497x; 1.0497x over previous
"""ArcticMoeBlock on 8 TRN2 NeuronCores — expert-parallel Bass kernel.

Reference computation (B=1, S=2048, H=1024, F=4096, E=8, TOPK=2):
    router_logits = x @ Wg                                   [S, E]  (fp32)
    top-2 softmax -> combine[s, e] (nonzero only for the 2 selected experts)
    per expert e: y_e = (silu(x @ W1[e]) * (x @ W3[e])) @ W2[e]
    final[s, :] = sum_e combine[s, e] * y_e[s, :]

Sharding: expert-parallel. Core e holds W1/W3/W2 of expert e (bf16,
host-packed for contiguous DMA), the token batch is replicated (passed
pre-transposed as xT so no on-device transposes are needed). Every core
computes the full fp32 router, selects its own combine column via a
one-hot vector, scales its expert output, and an AllReduce over the 8
cores produces the final output on every core.

Matmul layout notes (out = lhsT.T @ rhs, lhsT stationary [K<=128, M<=128]):
  h1T[f, s] = sum_h W1[h, f] xT[h, s]   lhsT = W1 tile (h-part), rhs = xT
  y[s, h]   = sum_f g[s, f] W2[f, h]    lhsT = gT tile (f-part), rhs = W2
so activations stay in [feature, token] layout until the W2 matmul flips
them back to natural [token, feature] — zero transposes anywhere.
"""

import numpy as np
import ml_dtypes

import concourse.bass as bass
import concourse.mybir as mybir
import concourse.tile as tile
from concourse import bacc
from concourse.bass_utils import run_bass_kernel_spmd

AF = mybir.ActivationFunctionType
ALU = mybir.AluOpType
AX = mybir.AxisListType
F32 = mybir.dt.float32
BF16 = mybir.dt.bfloat16

P = 128        # partition count
NF = 512       # matmul moving-operand chunk (one PSUM bank in fp32)
N_CORES = 8

# Set by test harness to capture profile info; harmless otherwise.
TRACE = False
LAST_RESULT = None

_compiled = {}


def _build(S, H, F, E, n_cores, s_blk):
    KH = H // P          # contraction tiles for W1/W3 (over hidden dim)
    MF = F // P          # f-tiles (output partitions of W1/W3, contraction of W2)
    NS = S // P          # s-tiles over the full sequence
    NBLK = S // s_blk    # token blocks
    MSB = s_blk // P     # s-tiles per block
    SFB = s_blk // NF    # 512-wide s-chunks per block (W1/W3 rhs)
    NH = H // NF         # 512-wide h-chunks (W2 rhs)
    NSE = NS * E
    assert NSE <= NF and MSB <= 8 and 2 * SFB <= 8

    nc = bacc.Bacc("TRN2", target_bir_lowering=False, debug=False,
                   num_devices=n_cores)

    xt32_e = nc.dram_tensor("xt_f32", [H, S], F32, kind="ExternalInput")
    xtbf_e = nc.dram_tensor("xt_bf16", [H, S], BF16, kind="ExternalInput")
    wg_e = nc.dram_tensor("wg", [H, E], F32, kind="ExternalInput")
    w1t_e = nc.dram_tensor("w1t", [MF, P, H], BF16, kind="ExternalInput")
    w3t_e = nc.dram_tensor("w3t", [MF, P, H], BF16, kind="ExternalInput")
    w2_e = nc.dram_tensor("w2", [F, H], BF16, kind="ExternalInput")
    esel_e = nc.dram_tensor("esel", [P, E], F32, kind="ExternalInput")
    outf_e = nc.dram_tensor("out_final", [S, H], F32, kind="ExternalOutput")
    outl_e = nc.dram_tensor("out_logits", [S, E], F32, kind="ExternalOutput")

    with tile.TileContext(nc) as tc:
        with (
            tc.tile_pool(name="persist", bufs=1) as pp,
            tc.tile_pool(name="xtbp", bufs=KH) as xtbp,
            tc.tile_pool(name="xfp", bufs=6) as xfp,
            tc.tile_pool(name="wgp", bufs=KH) as wgp,
            tc.tile_pool(name="w13p", bufs=3) as w13p,
            tc.tile_pool(name="w2p", bufs=4) as w2p,
            tc.tile_pool(name="gp", bufs=MF) as gp,
            tc.tile_pool(name="silup", bufs=4) as silup,
            tc.tile_pool(name="obp", bufs=4) as obp,
            tc.tile_pool(name="psp", bufs=8, space="PSUM") as psp,
            tc.tile_pool(name="dramp", bufs=1, space="DRAM") as dramp,
        ):
            # resident bf16 xT first: the FFN matmuls need all KH tiles
            # before their first MM, so get these DMAs queued ahead.
            xtb = []
            for k in range(KH):
                xk = xtbp.tile([P, S], BF16, tag="xtb", name=f"xtb{k}")
                nc.sync.dma_start(out=xk[:], in_=xtbf_e[k * P:(k + 1) * P, :])
                xtb.append(xk)

            # ---------------- router (fp32) ----------------
            # One PSUM bank per s-tile: a start=True matmul clears the whole
            # bank's accumulation state, so groups must not share a bank.
            wgs = []
            for k in range(KH):
                wgk = wgp.tile([P, E], F32, tag="wgk", name=f"wgk{k}")
                nc.sync.dma_start(out=wgk[:], in_=wg_e[k * P:(k + 1) * P, :])
                wgs.append(wgk)
            logits_sb = pp.tile([P, NSE], F32)
            for m in range(NS):
                ps_m = psp.tile([P, E], F32, tag="ps", name=f"psl{m}")
                for k in range(KH):
                    xf = xfp.tile([P, P], F32, tag="xf", name=f"xf_{m}_{k}")
                    nc.sync.dma_start(
                        out=xf[:],
                        in_=xt32_e[k * P:(k + 1) * P, m * P:(m + 1) * P])
                    nc.tensor.matmul(ps_m[:], xf[:], wgs[k][:],
                                     start=(k == 0), stop=(k == KH - 1))
                nc.vector.tensor_copy(logits_sb[:, m * E:(m + 1) * E], ps_m[:])
            nc.sync.dma_start(
                out=outl_e.rearrange("(m p) e -> p m e", p=P),
                in_=logits_sb[:].rearrange("p (m e) -> p m e", e=E),
            )

            # combine weights: top-2 softmax, select this core's expert col
            esel_sb = pp.tile([P, E], F32)
            nc.sync.dma_start(out=esel_sb[:], in_=esel_e[:])

            L3 = logits_sb[:].rearrange("p (m e) -> p m e", e=E)
            m1 = pp.tile([P, NS], F32)
            nc.vector.reduce_max(m1[:], L3, axis=AX.X)
            mask = pp.tile([P, NSE], F32)
            mask3 = mask[:].rearrange("p (m e) -> p m e", e=E)
            m1b = m1[:].unsqueeze(2).broadcast_to([P, NS, E])
            nc.vector.tensor_tensor(mask3, L3, m1b, op=ALU.is_equal)
            lm = pp.tile([P, NSE], F32)
            lm3 = lm[:].rearrange("p (m e) -> p m e", e=E)
            # (mask * -1e30) + logits: masked-out argmax -> -inf
            nc.vector.scalar_tensor_tensor(
                lm3, mask3, -1e30, L3, op0=ALU.mult, op1=ALU.add)
            m2 = pp.tile([P, NS], F32)
            nc.vector.reduce_max(m2[:], lm3, axis=AX.X)
            dm = pp.tile([P, NS], F32)
            nc.vector.tensor_sub(dm[:], m1[:], m2[:])
            w1w = pp.tile([P, NS], F32)
            nc.scalar.activation(w1w[:], dm[:], AF.Sigmoid)
            w2w = pp.tile([P, NS], F32)
            nc.vector.tensor_scalar(w2w[:], w1w[:], -1.0, 1.0,
                                    op0=ALU.mult, op1=ALU.add)
            lesel = pp.tile([P, NSE], F32)
            le3 = lesel[:].rearrange("p (m e) -> p m e", e=E)
            eselb = esel_sb[:].unsqueeze(1).broadcast_to([P, NS, E])
            nc.vector.tensor_tensor(le3, L3, eselb, op=ALU.mult)
            le = pp.tile([P, NS], F32)
            nc.vector.reduce_sum(le[:], le3, axis=AX.X)
            eq1 = pp.tile([P, NS], F32)
            nc.vector.tensor_tensor(eq1[:], le[:], m1[:], op=ALU.is_equal)
            eq2 = pp.tile([P, NS], F32)
            nc.vector.tensor_tensor(eq2[:], le[:], m2[:], op=ALU.is_equal)
            cq1 = pp.tile([P, NS], F32)
            nc.vector.tensor_tensor(cq1[:], eq1[:], w1w[:], op=ALU.mult)
            cq2 = pp.tile([P, NS], F32)
            nc.vector.tensor_tensor(cq2[:], eq2[:], w2w[:], op=ALU.mult)
            c_sb = pp.tile([P, NS], F32)
            nc.vector.tensor_add(c_sb[:], cq1[:], cq2[:])

            # per-(block, n-pass) AllReduce chunks so comm overlaps compute
            ar_ins = {}
            ar_outs = {}
            for b in range(NBLK):
                for n in range(NH):
                    ar_ins[b, n] = dramp.tile([s_blk, NF], F32,
                                              name=f"ar_in_{b}_{n}")
                    ar_outs[b, n] = dramp.tile([s_blk, NF], F32,
                                               addr_space="Shared",
                                               name=f"ar_out_{b}_{n}")

            # ---------------- expert FFN over token blocks ----------------
            for b in range(NBLK):
                bs0 = b * s_blk
                # h1T/h3T/gT for this block, f-tile by f-tile
                gms = []
                for m in range(MF):
                    w1m = w13p.tile([P, H], BF16, tag="w1m", name=f"w1m_{b}_{m}")
                    nc.sync.dma_start(out=w1m[:], in_=w1t_e[m, :, :])
                    w3m = w13p.tile([P, H], BF16, tag="w3m", name=f"w3m_{b}_{m}")
                    nc.sync.dma_start(out=w3m[:], in_=w3t_e[m, :, :])
                    ph1 = [psp.tile([P, NF], F32, tag="ps", name=f"ph1_{b}_{m}_{sf}")
                           for sf in range(SFB)]
                    ph3 = [psp.tile([P, NF], F32, tag="ps", name=f"ph3_{b}_{m}_{sf}")
                           for sf in range(SFB)]
                    for k in range(KH):
                        st, sp = (k == 0), (k == KH - 1)
                        for sf in range(SFB):
                            rhs = xtb[k][:, bs0 + sf * NF: bs0 + (sf + 1) * NF]
                            nc.tensor.matmul(ph1[sf][:], w1m[:, k * P:(k + 1) * P],
                                             rhs, start=st, stop=sp)
                        for sf in range(SFB):
                            rhs = xtb[k][:, bs0 + sf * NF: bs0 + (sf + 1) * NF]
                            nc.tensor.matmul(ph3[sf][:], w3m[:, k * P:(k + 1) * P],
                                             rhs, start=st, stop=sp)
                    gm = gp.tile([P, s_blk], BF16, tag="gm", name=f"gm_{b}_{m}")
                    for sf in range(SFB):
                        silu_t = silup.tile([P, NF], F32, tag="silu",
                                            name=f"silu_{b}_{m}_{sf}")
                        nc.scalar.activation(silu_t[:], ph1[sf][:], AF.Silu)
                        nc.vector.tensor_tensor(
                            gm[:, sf * NF:(sf + 1) * NF],
                            silu_t[:], ph3[sf][:], op=ALU.mult)
                    gms.append(gm)

                # y[s, h] = gT.T @ W2, k-outer so W2 streams exactly once
                for n in range(NH):
                    pso = [psp.tile([P, NF], F32, tag="ps", name=f"pso_{b}_{n}_{ms}")
                           for ms in range(MSB)]
                    for k in range(MF):
                        w2t = w2p.tile([P, NF], BF16, tag="w2t",
                                       name=f"w2t_{b}_{n}_{k}")
                        nc.sync.dma_start(
                            out=w2t[:],
                            in_=w2_e[k * P:(k + 1) * P, n * NF:(n + 1) * NF])
                        st, sp = (k == 0), (k == MF - 1)
                        for ms in range(MSB):
                            nc.tensor.matmul(
                                pso[ms][:],
                                gms[k][:, ms * P:(ms + 1) * P],
                                w2t[:], start=st, stop=sp)
                    for ms in range(MSB):
                        gs = b * MSB + ms
                        ob = obp.tile([P, NF], F32, tag="ob",
                                      name=f"ob_{b}_{n}_{ms}")
                        nc.vector.tensor_scalar_mul(
                            ob[:], pso[ms][:], c_sb[:, gs:gs + 1])
                        nc.sync.dma_start(
                            out=ar_ins[b, n][ms * P:(ms + 1) * P, :],
                            in_=ob[:])
                    # combine this chunk across experts while the next
                    # pass/block computes; only the last chunk is a tail
                    nc.gpsimd.collective_compute(
                        "AllReduce",
                        ALU.add,
                        replica_groups=[list(range(n_cores))],
                        ins=[ar_ins[b, n][:]],
                        outs=[ar_outs[b, n][:]],
                    )
                    nc.sync.dma_start(
                        out=outf_e[bs0:bs0 + s_blk, n * NF:(n + 1) * NF],
                        in_=ar_outs[b, n][:])

    nc.compile()
    return nc


def _get_compiled(S, H, F, E, n_cores, s_blk):
    key = (S, H, F, E, n_cores, s_blk)
    if key not in _compiled:
        _compiled[key] = _build(*key)
    return _compiled[key]


def _pack_w13(w, H, F):
    # [H, F] -> [F//P, P, H] with w_packed[m, p, k*P+f] = w[k*P+p, m*P+f]
    return np.ascontiguousarray(
        w.astype(ml_dtypes.bfloat16)
        .reshape(H // P, P, F // P, P)
        .transpose(2, 1, 0, 3)
        .reshape(F // P, P, H))


def kernel(x, Wg, W1, W3, W2, s_blk=1024):
    global LAST_RESULT
    x = np.asarray(x)
    Wg = np.asarray(Wg, dtype=np.float32)
    W1 = np.asarray(W1)
    W3 = np.asarray(W3)
    W2 = np.asarray(W2)
    B, S, H = x.shape
    E = Wg.shape[1]
    F = W1.shape[2]
    assert B == 1 and E == N_CORES

    xt = np.ascontiguousarray(x.reshape(S, H).T.astype(np.float32))
    xt_bf = xt.astype(ml_dtypes.bfloat16)

    nc = _get_compiled(S, H, F, E, N_CORES, s_blk)

    in_maps = []
    for e in range(N_CORES):
        esel = np.zeros((P, E), np.float32)
        esel[:, e] = 1.0
        in_maps.append({
            "xt_f32": xt,
            "xt_bf16": xt_bf,
            "wg": Wg,
            "w1t": _pack_w13(W1[e], H, F),
            "w3t": _pack_w13(W3[e], H, F),
            "w2": np.ascontiguousarray(W2[e].astype(ml_dtypes.bfloat16)),
            "esel": esel,
        })

    if TRACE:
        import profhook  # noqa: F401  (injects the axon NTFF hook)
    res = run_bass_kernel_spmd(nc, in_maps, core_ids=list(range(N_CORES)),
                               trace=TRACE)
    LAST_RESULT = res
    final = np.asarray(res.results[0]["out_final"],
                       dtype=np.float32).reshape(B, S, H)
    logits = np.asarray(res.results[0]["out_logits"],
                        dtype=np.float32).reshape(B, S, E)
    return final, logits


# revision 10
# speedup vs baseline: 1.1037x; 1.0514x over previous
"""ArcticMoeBlock on 8 TRN2 NeuronCores — expert-parallel Bass kernel.

Reference computation (B=1, S=2048, H=1024, F=4096, E=8, TOPK=2):
    router_logits = x @ Wg                                   [S, E]  (fp32)
    top-2 softmax -> combine[s, e] (nonzero only for the 2 selected experts)
    per expert e: y_e = (silu(x @ W1[e]) * (x @ W3[e])) @ W2[e]
    final[s, :] = sum_e combine[s, e] * y_e[s, :]

Sharding: expert-parallel. Core e holds W1/W3/W2 of expert e (bf16,
host-packed for contiguous DMA), the token batch is replicated (passed
pre-transposed as xT so no on-device transposes are needed). Every core
computes the full fp32 router, selects its own combine column via a
one-hot vector, scales its expert output, and an AllReduce over the 8
cores produces the final output on every core.

Matmul layout notes (out = lhsT.T @ rhs, lhsT stationary [K<=128, M<=128]):
  h1T[f, s] = sum_h W1[h, f] xT[h, s]   lhsT = W1 tile (h-part), rhs = xT
  y[s, h]   = sum_f g[s, f] W2[f, h]    lhsT = gT tile (f-part), rhs = W2
so activations stay in [feature, token] layout until the W2 matmul flips
them back to natural [token, feature] — zero transposes anywhere.
"""

import numpy as np
import ml_dtypes

import concourse.bass as bass
import concourse.mybir as mybir
import concourse.tile as tile
from concourse import bacc
from concourse.bass_utils import run_bass_kernel_spmd

AF = mybir.ActivationFunctionType
ALU = mybir.AluOpType
AX = mybir.AxisListType
F32 = mybir.dt.float32
BF16 = mybir.dt.bfloat16

P = 128        # partition count
NF = 512       # matmul moving-operand chunk (one PSUM bank in fp32)
N_CORES = 8

# Set by test harness to capture profile info; harmless otherwise.
TRACE = False
LAST_RESULT = None

_compiled = {}


def _build(S, H, F, E, n_cores, s_blk):
    KH = H // P          # contraction tiles for W1/W3 (over hidden dim)
    MF = F // P          # f-tiles (output partitions of W1/W3, contraction of W2)
    NS = S // P          # s-tiles over the full sequence
    NBLK = S // s_blk    # token blocks
    MSB = s_blk // P     # s-tiles per block
    SFB = s_blk // NF    # 512-wide s-chunks per block (W1/W3 rhs)
    NH = H // NF         # 512-wide h-chunks (W2 rhs)
    NSE = NS * E
    assert NSE <= NF and MSB <= 8 and 2 * SFB <= 8

    nc = bacc.Bacc("TRN2", target_bir_lowering=False, debug=False,
                   num_devices=n_cores)

    xt32_e = nc.dram_tensor("xt_f32", [H, S], F32, kind="ExternalInput")
    xtbf_e = nc.dram_tensor("xt_bf16", [H, S], BF16, kind="ExternalInput")
    wg_e = nc.dram_tensor("wg", [H, E], F32, kind="ExternalInput")
    w1t_e = nc.dram_tensor("w1t", [MF, P, H], BF16, kind="ExternalInput")
    w3t_e = nc.dram_tensor("w3t", [MF, P, H], BF16, kind="ExternalInput")
    w2_e = nc.dram_tensor("w2", [F, H], BF16, kind="ExternalInput")
    esel_e = nc.dram_tensor("esel", [P, E], F32, kind="ExternalInput")
    outf_e = nc.dram_tensor("out_final", [S, H], F32, kind="ExternalOutput")
    outl_e = nc.dram_tensor("out_logits", [S, E], F32, kind="ExternalOutput")

    with tile.TileContext(nc) as tc:
        with (
            tc.tile_pool(name="persist", bufs=1) as pp,
            tc.tile_pool(name="xtbp", bufs=KH) as xtbp,
            tc.tile_pool(name="xfp", bufs=6) as xfp,
            tc.tile_pool(name="wgp", bufs=KH) as wgp,
            tc.tile_pool(name="w13p", bufs=4) as w13p,
            tc.tile_pool(name="w2p", bufs=6) as w2p,
            tc.tile_pool(name="gp", bufs=MF) as gp,
            tc.tile_pool(name="silup", bufs=4) as silup,
            tc.tile_pool(name="obp", bufs=4) as obp,
            tc.tile_pool(name="psp", bufs=8, space="PSUM") as psp,
            tc.tile_pool(name="dramp", bufs=1, space="DRAM") as dramp,
        ):
            # ---------------- router (fp32) ----------------
            # One PSUM bank per s-tile: a start=True matmul clears the whole
            # bank's accumulation state, so groups must not share a bank.
            # The resident bf16 xT loads are interleaved into the router's
            # DMA stream so they're done by the time the FFN matmuls start
            # without delaying the router's first matmul.
            wgs = []
            for k in range(KH):
                wgk = wgp.tile([P, E], F32, tag="wgk", name=f"wgk{k}")
                nc.sync.dma_start(out=wgk[:], in_=wg_e[k * P:(k + 1) * P, :])
                wgs.append(wgk)
            xtb = []
            logits_sb = pp.tile([P, NSE], F32)
            for m in range(NS):
                ps_m = psp.tile([P, E], F32, tag="ps", name=f"psl{m}")
                for k in range(KH):
                    xf = xfp.tile([P, P], F32, tag="xf", name=f"xf_{m}_{k}")
                    nc.sync.dma_start(
                        out=xf[:],
                        in_=xt32_e[k * P:(k + 1) * P, m * P:(m + 1) * P])
                    nc.tensor.matmul(ps_m[:], xf[:], wgs[k][:],
                                     start=(k == 0), stop=(k == KH - 1))
                nc.vector.tensor_copy(logits_sb[:, m * E:(m + 1) * E], ps_m[:])
                while len(xtb) < KH and len(xtb) <= m * KH // NS:
                    k = len(xtb)
                    xk = xtbp.tile([P, S], BF16, tag="xtb", name=f"xtb{k}")
                    nc.sync.dma_start(out=xk[:],
                                      in_=xtbf_e[k * P:(k + 1) * P, :])
                    xtb.append(xk)
            nc.sync.dma_start(
                out=outl_e.rearrange("(m p) e -> p m e", p=P),
                in_=logits_sb[:].rearrange("p (m e) -> p m e", e=E),
            )

            # combine weights: top-2 softmax, select this core's expert col
            esel_sb = pp.tile([P, E], F32)
            nc.sync.dma_start(out=esel_sb[:], in_=esel_e[:])

            L3 = logits_sb[:].rearrange("p (m e) -> p m e", e=E)
            m1 = pp.tile([P, NS], F32)
            nc.vector.reduce_max(m1[:], L3, axis=AX.X)
            mask = pp.tile([P, NSE], F32)
            mask3 = mask[:].rearrange("p (m e) -> p m e", e=E)
            m1b = m1[:].unsqueeze(2).broadcast_to([P, NS, E])
            nc.vector.tensor_tensor(mask3, L3, m1b, op=ALU.is_equal)
            lm = pp.tile([P, NSE], F32)
            lm3 = lm[:].rearrange("p (m e) -> p m e", e=E)
            # (mask * -1e30) + logits: masked-out argmax -> -inf
            nc.vector.scalar_tensor_tensor(
                lm3, mask3, -1e30, L3, op0=ALU.mult, op1=ALU.add)
            m2 = pp.tile([P, NS], F32)
            nc.vector.reduce_max(m2[:], lm3, axis=AX.X)
            dm = pp.tile([P, NS], F32)
            nc.vector.tensor_sub(dm[:], m1[:], m2[:])
            w1w = pp.tile([P, NS], F32)
            nc.scalar.activation(w1w[:], dm[:], AF.Sigmoid)
            w2w = pp.tile([P, NS], F32)
            nc.vector.tensor_scalar(w2w[:], w1w[:], -1.0, 1.0,
                                    op0=ALU.mult, op1=ALU.add)
            lesel = pp.tile([P, NSE], F32)
            le3 = lesel[:].rearrange("p (m e) -> p m e", e=E)
            eselb = esel_sb[:].unsqueeze(1).broadcast_to([P, NS, E])
            nc.vector.tensor_tensor(le3, L3, eselb, op=ALU.mult)
            le = pp.tile([P, NS], F32)
            nc.vector.reduce_sum(le[:], le3, axis=AX.X)
            eq1 = pp.tile([P, NS], F32)
            nc.vector.tensor_tensor(eq1[:], le[:], m1[:], op=ALU.is_equal)
            eq2 = pp.tile([P, NS], F32)
            nc.vector.tensor_tensor(eq2[:], le[:], m2[:], op=ALU.is_equal)
            cq1 = pp.tile([P, NS], F32)
            nc.vector.tensor_tensor(cq1[:], eq1[:], w1w[:], op=ALU.mult)
            cq2 = pp.tile([P, NS], F32)
            nc.vector.tensor_tensor(cq2[:], eq2[:], w2w[:], op=ALU.mult)
            c_sb = pp.tile([P, NS], F32)
            nc.vector.tensor_add(c_sb[:], cq1[:], cq2[:])

            # 1MB AllReduce chunks (4 s-tiles x 512 cols) so comm overlaps
            # compute; only the last chunk is an exposed tail
            MSC = 4                      # s-tiles per AR chunk
            NCH = (MSB + MSC - 1) // MSC  # chunks per (block, n) pass
            ar_ins = {}
            ar_outs = {}
            for b in range(NBLK):
                for n in range(NH):
                    for h in range(NCH):
                        ar_ins[b, n, h] = dramp.tile(
                            [MSC * P, NF], F32, name=f"ar_in_{b}_{n}_{h}")
                        ar_outs[b, n, h] = dramp.tile(
                            [MSC * P, NF], F32, addr_space="Shared",
                            name=f"ar_out_{b}_{n}_{h}")

            # ---------------- expert FFN over token blocks ----------------
            for b in range(NBLK):
                bs0 = b * s_blk
                # h1T/h3T/gT for this block, f-tile by f-tile
                gms = []
                for m in range(MF):
                    w1m = w13p.tile([P, H], BF16, tag="w1m", name=f"w1m_{b}_{m}")
                    nc.sync.dma_start(out=w1m[:], in_=w1t_e[m, :, :])
                    w3m = w13p.tile([P, H], BF16, tag="w3m", name=f"w3m_{b}_{m}")
                    nc.sync.dma_start(out=w3m[:], in_=w3t_e[m, :, :])
                    ph1 = [psp.tile([P, NF], F32, tag="ps", name=f"ph1_{b}_{m}_{sf}")
                           for sf in range(SFB)]
                    ph3 = [psp.tile([P, NF], F32, tag="ps", name=f"ph3_{b}_{m}_{sf}")
                           for sf in range(SFB)]
                    for k in range(KH):
                        st, sp = (k == 0), (k == KH - 1)
                        for sf in range(SFB):
                            rhs = xtb[k][:, bs0 + sf * NF: bs0 + (sf + 1) * NF]
                            nc.tensor.matmul(ph1[sf][:], w1m[:, k * P:(k + 1) * P],
                                             rhs, start=st, stop=sp)
                        for sf in range(SFB):
                            rhs = xtb[k][:, bs0 + sf * NF: bs0 + (sf + 1) * NF]
                            nc.tensor.matmul(ph3[sf][:], w3m[:, k * P:(k + 1) * P],
                                             rhs, start=st, stop=sp)
                    gm = gp.tile([P, s_blk], BF16, tag="gm", name=f"gm_{b}_{m}")
                    for sf in range(SFB):
                        silu_t = silup.tile([P, NF], F32, tag="silu",
                                            name=f"silu_{b}_{m}_{sf}")
                        nc.scalar.activation(silu_t[:], ph1[sf][:], AF.Silu)
                        nc.vector.tensor_tensor(
                            gm[:, sf * NF:(sf + 1) * NF],
                            silu_t[:], ph3[sf][:], op=ALU.mult)
                    gms.append(gm)

                # y[s, h] = gT.T @ W2, k-outer so W2 streams exactly once
                for n in range(NH):
                    pso = [psp.tile([P, NF], F32, tag="ps", name=f"pso_{b}_{n}_{ms}")
                           for ms in range(MSB)]
                    for k in range(MF):
                        w2t = w2p.tile([P, NF], BF16, tag="w2t",
                                       name=f"w2t_{b}_{n}_{k}")
                        nc.sync.dma_start(
                            out=w2t[:],
                            in_=w2_e[k * P:(k + 1) * P, n * NF:(n + 1) * NF])
                        st, sp = (k == 0), (k == MF - 1)
                        for ms in range(MSB):
                            nc.tensor.matmul(
                                pso[ms][:],
                                gms[k][:, ms * P:(ms + 1) * P],
                                w2t[:], start=st, stop=sp)
                    for ms in range(MSB):
                        gs = b * MSB + ms
                        ob = obp.tile([P, NF], F32, tag="ob",
                                      name=f"ob_{b}_{n}_{ms}")
                        nc.vector.tensor_scalar_mul(
                            ob[:], pso[ms][:], c_sb[:, gs:gs + 1])
                        h, msl = divmod(ms, MSC)
                        nc.sync.dma_start(
                            out=ar_ins[b, n, h][msl * P:(msl + 1) * P, :],
                            in_=ob[:])
                        if msl == MSC - 1 or ms == MSB - 1:
                            # combine this chunk across experts while the
                            # next pass/block computes; only the last chunk
                            # is an exposed tail. The ar_out -> output copy
                            # rides the gpsimd queue: on sync's in-order
                            # queue its AR-wait would block the weight
                            # streams behind it and starve the PE.
                            nc.gpsimd.collective_compute(
                                "AllReduce",
                                ALU.add,
                                replica_groups=[list(range(n_cores))],
                                ins=[ar_ins[b, n, h][:]],
                                outs=[ar_outs[b, n, h][:]],
                            )
                            nc.gpsimd.dma_start(
                                out=outf_e[bs0 + h * MSC * P:
                                           bs0 + h * MSC * P + MSC * P,
                                           n * NF:(n + 1) * NF],
                                in_=ar_outs[b, n, h][:])

    nc.compile()
    return nc


def _get_compiled(S, H, F, E, n_cores, s_blk):
    key = (S, H, F, E, n_cores, s_blk)
    if key not in _compiled:
        _compiled[key] = _build(*key)
    return _compiled[key]


def _pack_w13(w, H, F):
    # [H, F] -> [F//P, P, H] with w_packed[m, p, k*P+f] = w[k*P+p, m*P+f]
    return np.ascontiguousarray(
        w.astype(ml_dtypes.bfloat16)
        .reshape(H // P, P, F // P, P)
        .transpose(2, 1, 0, 3)
        .reshape(F // P, P, H))


def kernel(x, Wg, W1, W3, W2, s_blk=1024):
    global LAST_RESULT
    x = np.asarray(x)
    Wg = np.asarray(Wg, dtype=np.float32)
    W1 = np.asarray(W1)
    W3 = np.asarray(W3)
    W2 = np.asarray(W2)
    B, S, H = x.shape
    E = Wg.shape[1]
    F = W1.shape[2]
    assert B == 1 and E == N_CORES

    xt = np.ascontiguousarray(x.reshape(S, H).T.astype(np.float32))
    xt_bf = xt.astype(ml_dtypes.bfloat16)

    nc = _get_compiled(S, H, F, E, N_CORES, s_blk)

    in_maps = []
    for e in range(N_CORES):
        esel = np.zeros((P, E), np.float32)
        esel[:, e] = 1.0
        in_maps.append({
            "xt_f32": xt,
            "xt_bf16": xt_bf,
            "wg": Wg,
            "w1t": _pack_w13(W1[e], H, F),
            "w3t": _pack_w13(W3[e], H, F),
            "w2": np.ascontiguousarray(W2[e].astype(ml_dtypes.bfloat16)),
            "esel": esel,
        })

    if TRACE:
        import profhook  # noqa: F401  (injects the axon NTFF hook)
    res = run_bass_kernel_spmd(nc, in_maps, core_ids=list(range(N_CORES)),
                               trace=TRACE)
    LAST_RESULT = res
    final = np.asarray(res.results[0]["out_final"],
                       dtype=np.float32).reshape(B, S, H)
    logits = np.asarray(res.results[0]["out_logits"],
                        dtype=np.float32).reshape(B, S, E)
    return final, logits
